# revision 10
# baseline (speedup 1.0000x reference)
"""Trainium2 Bass kernel for DigitConvolutionalModel (8-core data parallel).

Computation: x(B,784) -> 3x3 valid conv on 28x28 -> flatten(676)
             -> FC(100)+ReLU -> FC(10), B = 65536.

Algebraic restructure (host side, exact): the conv is linear, so conv and
fc1 fold into one 784->100 matrix W1eff (accumulated in float64). The
device kernel is then just two matmul layers per 512-sample tile:
  h = relu(x @ W1eff + b1);  y = h @ fc2_w.T + b2.

Numerics: the matmul datapath runs in fp16 (inputs rounded once on the
host); the output writeback is fp16 too (upcast on host). Measured
end-to-end scale-relative absmax error vs the fp32 reference ~9e-4.

Per-core layout (B_shard=8192 = 16 tiles x 512): x is pre-transposed on
the host to feature-major so the contraction lands on SBUF partitions.
The WHOLE shard (12.6 MB) is SBUF-resident: per tile one [128, 6*512]
fp16 slab (features 0..767), remainders packed in xr as in the baseline.
The x stream is 16 whole-tile DMAs balanced across the two HWDGE rings
(sync carries w1m+xr+consts then odd tiles; scalar carries even tiles,
tile 0 split in half so the PE can start early). With no buffer reuse
there are no WAR hazards: the SDMA engines stream HBM at line rate
start to finish, and the PE (2.06us/tile pair cadence) tracks ~1 tile
behind the DMA stream (2.2us/tile) instead of queueing behind an
oversized warmup.
"""

import numpy as np

import concourse.bass as bass
import concourse.mybir as mybir
import concourse.tile as tile
from concourse.bass_utils import run_bass_kernel_spmd
from concourse.vector_clock import ScopedClock

N_CORES = 8
B_TOTAL = 65536
B_SHARD = B_TOTAL // N_CORES  # 8192
BT = 512  # batch tile (one PSUM bank of fp32)
N_TILES = B_SHARD // BT  # 16
FC = 6  # full 128-partition feature chunks (6*128 = 768)
F_REM = 784 - FC * 128  # 16 remainder features
H1 = 100
H2 = 10
TW = FC * BT  # columns of one tile in the xm slab (3072)

_f32 = mybir.dt.float32
_f16 = mybir.dt.float16


class SplitDrainTileContext(tile.TileContext):
    """TileContext whose tail drain carries at most one sync wait.

    The pinned walrus rejects instructions with >2 sync waits
    ("Too many sync wait commands" in setupSyncWait); the stock tail
    drain accumulates one wait per active proc. Emit one drain per
    wait instead — consecutive drains on the sync engine are
    semantically equivalent to one drain carrying all the waits.
    """

    def _drain_and_barrier(self, tick_clock, wait_clock):
        nc = self.nc
        # Cheap tail: the stock version runs two full EVSEM butterflies
        # (~13us measured). Instead: gpsimd waits on the whole vector
        # clock (all tracked incs have landed), every engine drains its
        # own DGE queues, gpsimd clears the sem ranges, and one
        # sequencer-level sem-only barrier closes the kernel.
        drain_inst = nc.gpsimd.drain()
        wait_clock.add_sem_waits(
            drain_inst.ins, ScopedClock({None: tick_clock.global_clock})
        )
        raw = drain_inst.ins
        si = raw.sync_info
        if si is not None and si.on_wait and len(si.on_wait) > 1:
            waits = list(si.on_wait)
            si.on_wait = waits[:1]
            raw.sync_info = si
            for w in waits[1:]:
                extra = nc.gpsimd.drain()
                extra.ins.sync_info = mybir.SyncInfo(on_wait=[w], on_update=[])
        for eng in (nc.sync, nc.scalar, nc.vector, nc.tensor):
            eng.drain()

        # No tail barrier: gpsimd's global-clock waits above guarantee all
        # tracked sem incs (incl. DMA completions) have landed before the
        # clears, and NRT serializes re-executions on all-engine completion.
        assert self.sems is not None
        popped = nc._tile_sem_poison_stack.pop()
        assert popped is self._sem_poison
        nc.clear_and_free_semaphores(list(self.sems.allocated().values()))


def _split_sync_waits(nc: bass.Bass, limit: int = 1) -> None:
    """Walrus-compat post-pass: the pinned walrus rejects instructions
    carrying more than ~2 sync waits. Hoist excess waits onto NoOp
    instructions inserted just before the offending instruction on the
    same engine — semantically identical (waits run in stream order)."""
    n = 0
    for fn in nc.m.functions:
        for bb in fn.blocks:
            out = []
            changed = False
            for inst in bb.instructions:
                si = inst.sync_info
                if si is not None and si.on_wait and len(si.on_wait) > limit:
                    waits = list(si.on_wait)
                    for i in range(0, len(waits) - limit, limit):
                        nop = mybir.InstNoOp(
                            name=f"swsplit-{n}",
                            ins=[],
                            outs=[],
                            sync_info=mybir.SyncInfo(
                                on_wait=waits[i : i + limit], on_update=[]
                            ),
                        )
                        nop.engine = inst.engine
                        out.append(nop)
                        n += 1
                    si.on_wait = waits[len(waits) - limit :]
                    inst.sync_info = si
                    changed = True
                out.append(inst)
            if changed:
                bb.instructions = out
    return


CW = FC * H1 + H1 + H2 + 2  # packed consts cols: w1m 600 | w1r 100 | w2 10 | b1 1 | b2 1


def _build_nc(warm_iters: int = 30) -> bass.Bass:
    nc = bass.Bass(monotonic_sem_count=0)
    # whole shard, feature-major: column index = t*TW + c*BT + b
    xm = nc.dram_tensor("xm", [128, N_TILES * TW], _f16, kind="ExternalInput")
    # remainder features packed 4 tile-groups x 16 features into 128
    # partitions at 32-aligned offsets (PE row-group granularity)
    xr = nc.dram_tensor("xr", [128, 4 * BT], _f16, kind="ExternalInput")
    # all weights/biases packed in ONE tensor so the preload is cheap to
    # issue (each HWDGE issue costs ~0.6us of ring-issue time)
    cst = nc.dram_tensor("cst", [128, CW], _f16, kind="ExternalInput")
    y = nc.dram_tensor("y", [H2, N_TILES * BT], _f16, kind="ExternalOutput")

    with SplitDrainTileContext(nc) as tc:
        with (
            tc.tile_pool(name="consts", bufs=1) as cpool,
            tc.tile_pool(name="hp", bufs=4) as hpool,
            tc.tile_pool(name="psh", bufs=4, space="PSUM") as psh,
            tc.tile_pool(name="pso", bufs=3, space="PSUM") as pso,
            tc.tile_pool(name="wps", bufs=1, space="PSUM") as wpool,
        ):
            cst_sb = cpool.tile([128, CW], _f16, tag="cst")
            w1m_sb = cst_sb[:, : FC * H1]
            w1r_sb = cst_sb[:, FC * H1 : FC * H1 + H1]
            w2_sb = cst_sb[:H1, FC * H1 + H1 : FC * H1 + H1 + H2]
            # biases ride the f16 pack; DVE upcasts them to f32 once
            # (tensor_scalar requires f32 scalar operands)
            bias_sb = cpool.tile([H1, 2], _f32, tag="bias")
            b1_sb = bias_sb[:H1, 0:1]
            b2_sb = bias_sb[:H2, 1:2]
            xr_sb = cpool.tile([128, 4 * BT], _f16, tag="xr")
            # outputs accumulate here; tapered writeback
            o_sb = cpool.tile([H2, N_TILES * BT], _f16, tag="o")
            # the whole x shard lives in SBUF: no buffer reuse, no WAR
            # stalls; per-slice RAW deps gate each tile's matmuls.
            xm_sb = cpool.tile([128, N_TILES * TW], _f16, tag="xm")

            # DMA schedule: every tile is split across BOTH HWDGE rings —
            # chunks 0-2 (first 1536 cols) ride scalar (B), chunks 3-5 ride
            # sync (A) — so tile t's data completes at a uniform ~2.0us
            # cadence with no ring ever lagging the processing order. The
            # consts are split the same way: B leads with tile 0's first
            # half (the PE's first dependency after w1m chunk 0 in cstA).
            #   A (sync):   cstA, xrA(q=0,1), t0a, t1a, ..., t15a
            #   B (scalar): t0b, cstB, xrB(q=2,3), t1b, t2b, ..., t15b
            hw = TW // 2
            ch = CW // 2  # 356: w1m chunks 0..3 land in cstA
            nc.scalar.dma_start(out=xm_sb[:, :hw], in_=xm[:, :hw])
            nc.sync.dma_start(out=cst_sb[:, :ch], in_=cst[:, :ch])
            nc.sync.dma_start(out=xr_sb[:, : 2 * BT], in_=xr[:, : 2 * BT])
            nc.scalar.dma_start(out=cst_sb[:, ch:], in_=cst[:, ch:])
            nc.scalar.dma_start(out=xr_sb[:, 2 * BT :], in_=xr[:, 2 * BT :])
            nc.sync.dma_start(out=xm_sb[:, hw:TW], in_=xm[:, hw:TW])
            for t in range(1, N_TILES):
                c0 = t * TW
                nc.scalar.dma_start(
                    out=xm_sb[:, c0 : c0 + hw], in_=xm[:, c0 : c0 + hw]
                )
                nc.sync.dma_start(
                    out=xm_sb[:, c0 + hw : c0 + TW], in_=xm[:, c0 + hw : c0 + TW]
                )
            nc.vector.tensor_copy(bias_sb[:], cst_sb[:H1, CW - 2 : CW])

            # PE pre-warm while the first tile streams in (~2us): keeps the
            # clock/HAM state hot so the real stream runs at full rate.
            warm_sb = cpool.tile([128, 64], _f16, tag="warm")
            nc.vector.memset(warm_sb[:], 0)
            warm_ps = wpool.tile([64, 64], _f32, tag="wps")
            for _ in range(warm_iters):
                nc.tensor.matmul(
                    warm_ps[:], warm_sb[:, :64], warm_sb[:, :64], start=True, stop=True
                )

            # tile-serial stream: LDWEIGHTS is emitted per-matmul anyway
            # (no stationary reuse in the ISA), so pairing tiles buys
            # nothing — single-tile processing tracks the DMA stream with
            # the lowest latency.
            for t in range(N_TILES):
                xs_ = xm_sb[:, t * TW : (t + 1) * TW]
                g, q = t // 4, t % 4
                ph = psh.tile([H1, BT], _f32, tag="ph", name="ph")
                for c in range(FC):
                    nc.tensor.matmul(
                        ph[:],
                        w1m_sb[:, c * H1 : (c + 1) * H1],
                        xs_[:, c * BT : (c + 1) * BT],
                        start=(c == 0),
                        stop=False,
                    )
                nc.tensor.matmul(
                    ph[:],
                    w1r_sb[32 * g : 32 * g + F_REM, :],
                    xr_sb[32 * g : 32 * g + F_REM, q * BT : (q + 1) * BT],
                    start=False,
                    stop=True,
                    tile_position=(96, 0) if g == 3 else None,
                )

                # relu(ph + b1) on DVE — ACT stays a pure DMA-issue engine so
                # its HWDGE ring never stalls behind compute
                h = hpool.tile([H1, BT], _f16, tag="h", name="h")
                nc.vector.tensor_scalar(
                    h[:],
                    ph[:],
                    b1_sb[:, 0:1],
                    0.0,
                    mybir.AluOpType.add,
                    mybir.AluOpType.max,
                )

                po = pso.tile([H2, BT], _f32, tag="po", name="po")
                nc.tensor.matmul(po[:], w2_sb[:], h[:], start=True, stop=True)

                nc.vector.tensor_scalar_add(
                    o_sb[:, t * BT : (t + 1) * BT], po[:], b2_sb[:, 0:1]
                )
                # tapered writeback: big chunks leave mid-kernel, the
                # final write after the last tile is two tiles (20 KB f16)
                flush = {7: [(0, 8)], 11: [(8, 12)], 13: [(12, 14)],
                         15: [(14, 16)]}
                for i, (a, b) in enumerate(flush.get(t, [])):
                    eng = nc.sync if (t + i) % 2 == 1 else nc.scalar
                    eng.dma_start(
                        out=y[:, a * BT : b * BT], in_=o_sb[:, a * BT : b * BT]
                    )

    _split_sync_waits(nc)
    return nc


def _fold_conv_fc1(conv_w: np.ndarray, fc1_w: np.ndarray) -> np.ndarray:
    """Fold the 3x3 valid conv into fc1: W1eff[784, 100] such that
    h = x @ W1eff  ==  fc1( flatten(conv(x)) ).  Accumulated in float64."""
    F = fc1_w.astype(np.float64).T.reshape(26, 26, H1)
    W = np.zeros((28, 28, H1), np.float64)
    cw = conv_w.astype(np.float64)
    for di in range(3):
        for dj in range(3):
            W[di : di + 26, dj : dj + 26, :] += cw[di, dj] * F
    return W.reshape(784, H1).astype(np.float32)


def _make_in_maps(x, conv_w, fc1_w, fc1_b, fc2_w, fc2_b):
    w1eff = _fold_conv_fc1(conv_w, fc1_w)
    w1m = np.ascontiguousarray(
        w1eff[: FC * 128]
        .astype(np.float16)
        .reshape(FC, 128, H1)
        .transpose(1, 0, 2)
        .reshape(128, FC * H1)
    )
    w1r = np.zeros((128, H1), np.float16)
    for g in range(4):
        w1r[32 * g : 32 * g + F_REM] = w1eff[FC * 128 :].astype(np.float16)
    cst = np.zeros((128, CW), np.float16)
    cst[:, : FC * H1] = w1m
    cst[:, FC * H1 : FC * H1 + H1] = w1r
    cst[:H1, FC * H1 + H1 : FC * H1 + H1 + H2] = fc2_w.T.astype(np.float16)
    cst[:H1, CW - 2] = fc1_b.astype(np.float16)
    cst[:H2, CW - 1] = fc2_b.astype(np.float16)

    in_maps = []
    for s in range(N_CORES):
        xs = x[s * B_SHARD : (s + 1) * B_SHARD].reshape(N_TILES, BT, 784)
        # xm[p, t*TW + c*BT + b] = xs[t, b, c*128 + p]
        xm = np.ascontiguousarray(
            xs[:, :, : FC * 128]
            .astype(np.float16)
            .reshape(N_TILES, BT, FC, 128)
            .transpose(3, 0, 2, 1)
            .reshape(128, N_TILES * TW)
        )
        xr_flat = xs.reshape(B_SHARD, 784)[:, FC * 128 :].astype(np.float16)
        xr = np.zeros((128, 4 * BT), np.float16)
        for t in range(N_TILES):
            g, q = t // 4, t % 4
            xr[32 * g : 32 * g + F_REM, q * BT : (q + 1) * BT] = xr_flat[
                t * BT : (t + 1) * BT
            ].T
        in_maps.append({"xm": xm, "xr": xr, "cst": cst})
    return in_maps


def _gather(results) -> np.ndarray:
    out = np.empty((B_TOTAL, H2), np.float32)
    for s in range(N_CORES):
        ys = results[s]["y"]  # [H2, B_SHARD] f16
        out[s * B_SHARD : (s + 1) * B_SHARD] = ys.T.astype(np.float32)
    return out


def kernel_run(inputs: dict, trace: bool = False):
    """Run the kernel; returns (full output (65536,10) f32, BassKernelResults)."""
    x = np.ascontiguousarray(np.asarray(inputs["x"], dtype=np.float32))
    assert x.shape == (B_TOTAL, 784), x.shape
    in_maps = _make_in_maps(
        x,
        np.asarray(inputs["conv_w"], np.float32),
        np.asarray(inputs["fc1_w"], np.float32),
        np.asarray(inputs["fc1_b"], np.float32),
        np.asarray(inputs["fc2_w"], np.float32),
        np.asarray(inputs["fc2_b"], np.float32),
    )
    nc = _build_nc()
    res = run_bass_kernel_spmd(nc, in_maps, core_ids=list(range(N_CORES)), trace=trace)
    return _gather(res.results), res


def kernel(**inputs) -> np.ndarray:
    out, _ = kernel_run(inputs)
    return out


# revision 19
# speedup vs baseline: 1.0063x; 1.0063x over previous
"""Trainium2 Bass kernel for DigitConvolutionalModel (8-core data parallel).

Computation: x(B,784) -> 3x3 valid conv on 28x28 -> flatten(676)
             -> FC(100)+ReLU -> FC(10), B = 65536.

Algebraic restructure (host side, exact): the conv is linear, so conv and
fc1 fold into one 784->100 matrix W1eff (accumulated in float64). The
device kernel is then just two matmul layers per 512-sample tile:
  h = relu(x @ W1eff + b1);  y = h @ fc2_w.T + b2.

Numerics: the matmul datapath runs in fp16 (inputs rounded once on the
host); the output writeback is fp16 too (upcast on host). Measured
end-to-end scale-relative absmax error vs the fp32 reference ~9e-4.

Per-core layout (B_shard=8192 = 16 tiles x 512): x is pre-transposed on
the host to feature-major so the contraction lands on SBUF partitions.
The WHOLE shard (12.6 MB) is SBUF-resident: per tile one [128, 6*512]
fp16 slab (features 0..767), remainders packed in xr as in the baseline.
The x stream is 16 whole-tile DMAs balanced across the two HWDGE rings
(sync carries w1m+xr+consts then odd tiles; scalar carries even tiles,
tile 0 split in half so the PE can start early). With no buffer reuse
there are no WAR hazards: the SDMA engines stream HBM at line rate
start to finish, and the PE (2.06us/tile pair cadence) tracks ~1 tile
behind the DMA stream (2.2us/tile) instead of queueing behind an
oversized warmup.
"""

import numpy as np

import concourse.bass as bass
import concourse.mybir as mybir
import concourse.tile as tile
from concourse.bass_utils import run_bass_kernel_spmd
from concourse.vector_clock import ScopedClock

N_CORES = 8
B_TOTAL = 65536
B_SHARD = B_TOTAL // N_CORES  # 8192
BT = 512  # batch tile (one PSUM bank of fp32)
N_TILES = B_SHARD // BT  # 16
FC = 6  # full 128-partition feature chunks (6*128 = 768)
F_REM = 784 - FC * 128  # 16 remainder features
H1 = 100
H2 = 10
TW = FC * BT  # columns of one tile in the xm slab (3072)

_f32 = mybir.dt.float32
_f16 = mybir.dt.float16


class SplitDrainTileContext(tile.TileContext):
    """TileContext whose tail drain carries at most one sync wait.

    The pinned walrus rejects instructions with >2 sync waits
    ("Too many sync wait commands" in setupSyncWait); the stock tail
    drain accumulates one wait per active proc. Emit one drain per
    wait instead — consecutive drains on the sync engine are
    semantically equivalent to one drain carrying all the waits.
    """

    def _drain_and_barrier(self, tick_clock, wait_clock):
        nc = self.nc
        # Cheap tail: the stock version runs two full EVSEM butterflies
        # (~13us measured). Instead: gpsimd waits on the whole vector
        # clock (all tracked incs have landed), every engine drains its
        # own DGE queues, gpsimd clears the sem ranges, and one
        # sequencer-level sem-only barrier closes the kernel.
        drain_inst = nc.gpsimd.drain()
        wait_clock.add_sem_waits(
            drain_inst.ins, ScopedClock({None: tick_clock.global_clock})
        )
        raw = drain_inst.ins
        si = raw.sync_info
        if si is not None and si.on_wait and len(si.on_wait) > 1:
            waits = list(si.on_wait)
            si.on_wait = waits[:1]
            raw.sync_info = si
            for w in waits[1:]:
                extra = nc.gpsimd.drain()
                extra.ins.sync_info = mybir.SyncInfo(on_wait=[w], on_update=[])
        for eng in (nc.sync, nc.scalar, nc.vector, nc.tensor):
            eng.drain()

        # No tail barrier: gpsimd's global-clock waits above guarantee all
        # tracked sem incs (incl. DMA completions) have landed before the
        # clears, and NRT serializes re-executions on all-engine completion.
        assert self.sems is not None
        popped = nc._tile_sem_poison_stack.pop()
        assert popped is self._sem_poison
        nc.clear_and_free_semaphores(list(self.sems.allocated().values()))


def _split_sync_waits(nc: bass.Bass, limit: int = 1) -> None:
    """Walrus-compat post-pass: the pinned walrus rejects instructions
    carrying more than ~2 sync waits. Hoist excess waits onto NoOp
    instructions inserted just before the offending instruction on the
    same engine — semantically identical (waits run in stream order)."""
    n = 0
    for fn in nc.m.functions:
        for bb in fn.blocks:
            out = []
            changed = False
            for inst in bb.instructions:
                si = inst.sync_info
                if si is not None and si.on_wait and len(si.on_wait) > limit:
                    waits = list(si.on_wait)
                    for i in range(0, len(waits) - limit, limit):
                        nop = mybir.InstNoOp(
                            name=f"swsplit-{n}",
                            ins=[],
                            outs=[],
                            sync_info=mybir.SyncInfo(
                                on_wait=waits[i : i + limit], on_update=[]
                            ),
                        )
                        nop.engine = inst.engine
                        out.append(nop)
                        n += 1
                    si.on_wait = waits[len(waits) - limit :]
                    inst.sync_info = si
                    changed = True
                out.append(inst)
            if changed:
                bb.instructions = out
    return


# packed consts cols: w1m 600 | w1r lo 100 | w1r hi 100 | w2 10 | b1 1 | b2 1
CW = FC * H1 + 2 * H1 + H2 + 2


def _build_nc(warm_iters: int = 30) -> bass.Bass:
    nc = bass.Bass(monotonic_sem_count=0)
    # whole shard, feature-major: column index = t*TW + c*BT + b
    xm = nc.dram_tensor("xm", [128, N_TILES * TW], _f16, kind="ExternalInput")
    # remainder features, tight-packed: band 0 (rows 0-31) holds groups
    # 0 (rows 0-15) and 1 (rows 16-31); band 1 (rows 32-63) holds groups
    # 2 and 3. The unwanted half of each band is masked by zeros in the
    # stationary (w1r lo/hi variants), so no padding bytes ride the DMA.
    xr = nc.dram_tensor("xr", [64, 4 * BT], _f16, kind="ExternalInput")
    # all weights/biases packed in ONE tensor so the preload is cheap to
    # issue (each HWDGE issue costs ~0.6us of ring-issue time)
    cst = nc.dram_tensor("cst", [128, CW], _f16, kind="ExternalInput")
    y = nc.dram_tensor("y", [H2, N_TILES * BT], _f16, kind="ExternalOutput")

    with SplitDrainTileContext(nc) as tc:
        with (
            tc.tile_pool(name="consts", bufs=1) as cpool,
            tc.tile_pool(name="hp", bufs=4) as hpool,
            tc.tile_pool(name="psh", bufs=4, space="PSUM") as psh,
            tc.tile_pool(name="pso", bufs=3, space="PSUM") as pso,
            tc.tile_pool(name="wps", bufs=1, space="PSUM") as wpool,
        ):
            cst_sb = cpool.tile([128, CW], _f16, tag="cst")
            w1m_sb = cst_sb[:, : FC * H1]
            # two masked variants, each replicated at both 32-row bands so
            # weights and fmap share a start partition (walrus requires it):
            # lo cols: [w1rem; 0] pattern, hi cols: [0; w1rem]
            w1r_sb = cst_sb[:, FC * H1 : FC * H1 + 2 * H1]
            w2_sb = cst_sb[:H1, FC * H1 + 2 * H1 : FC * H1 + 2 * H1 + H2]
            # biases ride the f16 pack; DVE upcasts them to f32 once
            # (tensor_scalar requires f32 scalar operands)
            bias_sb = cpool.tile([H1, 2], _f32, tag="bias")
            b1_sb = bias_sb[:H1, 0:1]
            b2_sb = bias_sb[:H2, 1:2]
            xr_sb = cpool.tile([64, 4 * BT], _f16, tag="xr")
            # outputs accumulate here; tapered writeback
            o_sb = cpool.tile([H2, N_TILES * BT], _f16, tag="o")
            # the whole x shard lives in SBUF: no buffer reuse, no WAR
            # stalls; per-slice RAW deps gate each tile's matmuls.
            xm_sb = cpool.tile([128, N_TILES * TW], _f16, tag="xm")

            # DMA schedule: whole-tile transfers (786 KB; smaller DMAs
            # measurably drop aggregate ring bandwidth ~25%), alternating
            # rings so tiles land in processing order at ~1.9us cadence:
            #   A (sync):   cst, t1, xr_b0, t3, xr_b1, t5, t7, ..., t15
            #   B (scalar): t0, t2, t4, ..., t14
            # xr bands are slotted just-in-time: band 0 (tiles 0-7) right
            # after t1, band 1 (tiles 8-15) after t3.
            nc.scalar.dma_start(out=xm_sb[:, :TW], in_=xm[:, :TW])
            nc.sync.dma_start(out=cst_sb[:], in_=cst[:])
            for t in range(1, N_TILES):
                c0 = t * TW
                eng = nc.sync if t % 2 == 1 else nc.scalar
                eng.dma_start(out=xm_sb[:, c0 : c0 + TW], in_=xm[:, c0 : c0 + TW])
                if t == 1:
                    nc.sync.dma_start(
                        out=xr_sb[:32, :], in_=xr[:32, :]
                    )
                elif t == 3:
                    nc.sync.dma_start(
                        out=xr_sb[32:, :], in_=xr[32:, :]
                    )
            nc.vector.tensor_copy(bias_sb[:], cst_sb[:H1, CW - 2 : CW])

            # PE pre-warm while the first tile streams in (~2us): keeps the
            # clock/HAM state hot so the real stream runs at full rate.
            warm_sb = cpool.tile([128, 64], _f16, tag="warm")
            nc.vector.memset(warm_sb[:], 0)
            warm_ps = wpool.tile([64, 64], _f32, tag="wps")
            for _ in range(warm_iters):
                nc.tensor.matmul(
                    warm_ps[:], warm_sb[:, :64], warm_sb[:, :64], start=True, stop=True
                )

            # tile-serial stream: LDWEIGHTS is emitted per-matmul anyway
            # (no stationary reuse in the ISA), so pairing tiles buys
            # nothing — single-tile processing tracks the DMA stream with
            # the lowest latency.
            for t in range(N_TILES):
                xs_ = xm_sb[:, t * TW : (t + 1) * TW]
                g, q = t // 4, t % 4
                band, variant = g // 2, g % 2
                ph = psh.tile([H1, BT], _f32, tag="ph", name="ph")
                for c in range(FC):
                    nc.tensor.matmul(
                        ph[:],
                        w1m_sb[:, c * H1 : (c + 1) * H1],
                        xs_[:, c * BT : (c + 1) * BT],
                        start=(c == 0),
                        stop=False,
                    )
                # tight-packed remainder: the 32-row band holds two groups;
                # the w1r variant zero-masks the half that isn't group g.
                nc.tensor.matmul(
                    ph[:],
                    w1r_sb[32 * band : 32 * band + 32, variant * H1 : variant * H1 + H1],
                    xr_sb[32 * band : 32 * band + 32, q * BT : (q + 1) * BT],
                    start=False,
                    stop=True,
                    tile_position=(32 * band, 0),
                )

                # relu(ph + b1) on DVE — ACT stays a pure DMA-issue engine so
                # its HWDGE ring never stalls behind compute
                h = hpool.tile([H1, BT], _f16, tag="h", name="h")
                nc.vector.tensor_scalar(
                    h[:],
                    ph[:],
                    b1_sb[:, 0:1],
                    0.0,
                    mybir.AluOpType.add,
                    mybir.AluOpType.max,
                )

                po = pso.tile([H2, BT], _f32, tag="po", name="po")
                nc.tensor.matmul(po[:], w2_sb[:], h[:], start=True, stop=True)

                nc.vector.tensor_scalar_add(
                    o_sb[:, t * BT : (t + 1) * BT], po[:], b2_sb[:, 0:1]
                )
                # tapered writeback: big chunks leave mid-kernel, the
                # final write after the last tile is two tiles (20 KB f16)
                flush = {7: [(0, 8)], 11: [(8, 12)], 13: [(12, 14)],
                         15: [(14, 16)]}
                for i, (a, b) in enumerate(flush.get(t, [])):
                    eng = nc.sync if (t + i) % 2 == 1 else nc.scalar
                    eng.dma_start(
                        out=y[:, a * BT : b * BT], in_=o_sb[:, a * BT : b * BT]
                    )

    _split_sync_waits(nc)
    return nc


def _fold_conv_fc1(conv_w: np.ndarray, fc1_w: np.ndarray) -> np.ndarray:
    """Fold the 3x3 valid conv into fc1: W1eff[784, 100] such that
    h = x @ W1eff  ==  fc1( flatten(conv(x)) ).  Accumulated in float64."""
    F = fc1_w.astype(np.float64).T.reshape(26, 26, H1)
    W = np.zeros((28, 28, H1), np.float64)
    cw = conv_w.astype(np.float64)
    for di in range(3):
        for dj in range(3):
            W[di : di + 26, dj : dj + 26, :] += cw[di, dj] * F
    return W.reshape(784, H1).astype(np.float32)


def _make_in_maps(x, conv_w, fc1_w, fc1_b, fc2_w, fc2_b):
    w1eff = _fold_conv_fc1(conv_w, fc1_w)
    w1m = np.ascontiguousarray(
        w1eff[: FC * 128]
        .astype(np.float16)
        .reshape(FC, 128, H1)
        .transpose(1, 0, 2)
        .reshape(128, FC * H1)
    )
    # lo cols: w1rem at band rows 0-15 (zeros at 16-31); hi cols: w1rem at
    # band rows 16-31; both replicated at bands 0-31 and 32-63
    w1rem = w1eff[FC * 128 :].astype(np.float16)
    w1r = np.zeros((128, 2 * H1), np.float16)
    for band in range(2):
        w1r[32 * band : 32 * band + F_REM, :H1] = w1rem
        w1r[32 * band + F_REM : 32 * band + 32, H1:] = w1rem
    cst = np.zeros((128, CW), np.float16)
    cst[:, : FC * H1] = w1m
    cst[:, FC * H1 : FC * H1 + 2 * H1] = w1r
    cst[:H1, FC * H1 + 2 * H1 : FC * H1 + 2 * H1 + H2] = fc2_w.T.astype(np.float16)
    cst[:H1, CW - 2] = fc1_b.astype(np.float16)
    cst[:H2, CW - 1] = fc2_b.astype(np.float16)

    in_maps = []
    for s in range(N_CORES):
        xs = x[s * B_SHARD : (s + 1) * B_SHARD].reshape(N_TILES, BT, 784)
        # xm[p, t*TW + c*BT + b] = xs[t, b, c*128 + p]
        xm = np.ascontiguousarray(
            xs[:, :, : FC * 128]
            .astype(np.float16)
            .reshape(N_TILES, BT, FC, 128)
            .transpose(3, 0, 2, 1)
            .reshape(128, N_TILES * TW)
        )
        xr_flat = xs.reshape(B_SHARD, 784)[:, FC * 128 :].astype(np.float16)
        # tight pack: band b holds groups 2b (rows 0-15 of the band) and
        # 2b+1 (rows 16-31); columns index the group's 4 tiles
        xr = np.zeros((64, 4 * BT), np.float16)
        for t in range(N_TILES):
            g, q = t // 4, t % 4
            r0 = 32 * (g // 2) + F_REM * (g % 2)
            xr[r0 : r0 + F_REM, q * BT : (q + 1) * BT] = xr_flat[
                t * BT : (t + 1) * BT
            ].T
        in_maps.append({"xm": xm, "xr": xr, "cst": cst})
    return in_maps


def _gather(results) -> np.ndarray:
    out = np.empty((B_TOTAL, H2), np.float32)
    for s in range(N_CORES):
        ys = results[s]["y"]  # [H2, B_SHARD] f16
        out[s * B_SHARD : (s + 1) * B_SHARD] = ys.T.astype(np.float32)
    return out


def kernel_run(inputs: dict, trace: bool = False):
    """Run the kernel; returns (full output (65536,10) f32, BassKernelResults)."""
    x = np.ascontiguousarray(np.asarray(inputs["x"], dtype=np.float32))
    assert x.shape == (B_TOTAL, 784), x.shape
    in_maps = _make_in_maps(
        x,
        np.asarray(inputs["conv_w"], np.float32),
        np.asarray(inputs["fc1_w"], np.float32),
        np.asarray(inputs["fc1_b"], np.float32),
        np.asarray(inputs["fc2_w"], np.float32),
        np.asarray(inputs["fc2_b"], np.float32),
    )
    nc = _build_nc()
    res = run_bass_kernel_spmd(nc, in_maps, core_ids=list(range(N_CORES)), trace=trace)
    return _gather(res.results), res


def kernel(**inputs) -> np.ndarray:
    out, _ = kernel_run(inputs)
    return out


# revision 20
# speedup vs baseline: 1.0617x; 1.0551x over previous
"""Trainium2 Bass kernel for DigitConvolutionalModel (8-core data parallel).

Computation: x(B,784) -> 3x3 valid conv on 28x28 -> flatten(676)
             -> FC(100)+ReLU -> FC(10), B = 65536.

Algebraic restructure (host side, exact): the conv is linear, so conv and
fc1 fold into one 784->100 matrix W1eff (accumulated in float64). The
device kernel is then just two matmul layers per 512-sample tile:
  h = relu(x @ W1eff + b1);  y = h @ fc2_w.T + b2.

Numerics: the matmul datapath runs in fp16 (inputs rounded once on the
host); the output writeback is fp16 too (upcast on host). Measured
end-to-end scale-relative absmax error vs the fp32 reference ~9e-4.

Per-core layout (B_shard=8192 = 16 tiles x 512): x is pre-transposed on
the host to feature-major so the contraction lands on SBUF partitions.
The WHOLE shard (12.6 MB) is SBUF-resident: per tile one [128, 6*512]
fp16 slab (features 0..767), remainders packed in xr as in the baseline.
The x stream is 16 whole-tile DMAs balanced across the two HWDGE rings
(sync carries w1m+xr+consts then odd tiles; scalar carries even tiles,
tile 0 split in half so the PE can start early). With no buffer reuse
there are no WAR hazards: the SDMA engines stream HBM at line rate
start to finish, and the PE (2.06us/tile pair cadence) tracks ~1 tile
behind the DMA stream (2.2us/tile) instead of queueing behind an
oversized warmup.
"""

import numpy as np

import concourse.bass as bass
import concourse.mybir as mybir
import concourse.tile as tile
from concourse.bass_utils import run_bass_kernel_spmd
from concourse.vector_clock import ScopedClock

N_CORES = 8
B_TOTAL = 65536
B_SHARD = B_TOTAL // N_CORES  # 8192
BT = 512  # batch tile (one PSUM bank of fp32)
N_TILES = B_SHARD // BT  # 16
FC = 6  # full 128-partition feature chunks (6*128 = 768)
F_REM = 784 - FC * 128  # 16 remainder features
H1 = 100
H2 = 10
TW = FC * BT  # columns of one tile in the xm slab (3072)

_f32 = mybir.dt.float32
_f16 = mybir.dt.float16


class SplitDrainTileContext(tile.TileContext):
    """TileContext whose tail drain carries at most one sync wait.

    The pinned walrus rejects instructions with >2 sync waits
    ("Too many sync wait commands" in setupSyncWait); the stock tail
    drain accumulates one wait per active proc. Emit one drain per
    wait instead — consecutive drains on the sync engine are
    semantically equivalent to one drain carrying all the waits.
    """

    def _drain_and_barrier(self, tick_clock, wait_clock):
        nc = self.nc
        # Cheap tail: the stock version runs two full EVSEM butterflies
        # (~13us measured). Instead: gpsimd waits on the whole vector
        # clock (all tracked incs have landed), every engine drains its
        # own DGE queues, gpsimd clears the sem ranges, and one
        # sequencer-level sem-only barrier closes the kernel.
        drain_inst = nc.gpsimd.drain()
        wait_clock.add_sem_waits(
            drain_inst.ins, ScopedClock({None: tick_clock.global_clock})
        )
        raw = drain_inst.ins
        si = raw.sync_info
        if si is not None and si.on_wait and len(si.on_wait) > 1:
            waits = list(si.on_wait)
            si.on_wait = waits[:1]
            raw.sync_info = si
            for w in waits[1:]:
                extra = nc.gpsimd.drain()
                extra.ins.sync_info = mybir.SyncInfo(on_wait=[w], on_update=[])
        for eng in (nc.sync, nc.scalar, nc.vector, nc.tensor):
            eng.drain()

        # No tail barrier: gpsimd's global-clock waits above guarantee all
        # tracked sem incs (incl. DMA completions) have landed before the
        # clears, and NRT serializes re-executions on all-engine completion.
        assert self.sems is not None
        popped = nc._tile_sem_poison_stack.pop()
        assert popped is self._sem_poison
        nc.clear_and_free_semaphores(list(self.sems.allocated().values()))


def _split_sync_waits(nc: bass.Bass, limit: int = 1) -> None:
    """Walrus-compat post-pass: the pinned walrus rejects instructions
    carrying more than ~2 sync waits. Hoist excess waits onto NoOp
    instructions inserted just before the offending instruction on the
    same engine — semantically identical (waits run in stream order)."""
    n = 0
    for fn in nc.m.functions:
        for bb in fn.blocks:
            out = []
            changed = False
            for inst in bb.instructions:
                si = inst.sync_info
                if si is not None and si.on_wait and len(si.on_wait) > limit:
                    waits = list(si.on_wait)
                    for i in range(0, len(waits) - limit, limit):
                        nop = mybir.InstNoOp(
                            name=f"swsplit-{n}",
                            ins=[],
                            outs=[],
                            sync_info=mybir.SyncInfo(
                                on_wait=waits[i : i + limit], on_update=[]
                            ),
                        )
                        nop.engine = inst.engine
                        out.append(nop)
                        n += 1
                    si.on_wait = waits[len(waits) - limit :]
                    inst.sync_info = si
                    changed = True
                out.append(inst)
            if changed:
                bb.instructions = out
    return


# packed consts cols: w1m 600 | w1r lo 100 | w1r hi 100 | w2 10 | b1 1 | b2 1
CW = FC * H1 + 2 * H1 + H2 + 2


def _build_nc(warm_iters: int = 30) -> bass.Bass:
    nc = bass.Bass(monotonic_sem_count=0)
    # whole shard, feature-major: column index = t*TW + c*BT + b
    xm = nc.dram_tensor("xm", [128, N_TILES * TW], _f16, kind="ExternalInput")
    # remainder features, tight-packed: band 0 (rows 0-31) holds groups
    # 0 (rows 0-15) and 1 (rows 16-31); band 1 (rows 32-63) holds groups
    # 2 and 3. The unwanted half of each band is masked by zeros in the
    # stationary (w1r lo/hi variants), so no padding bytes ride the DMA.
    xr = nc.dram_tensor("xr", [64, 4 * BT], _f16, kind="ExternalInput")
    # all weights/biases packed in ONE tensor so the preload is cheap to
    # issue (each HWDGE issue costs ~0.6us of ring-issue time)
    cst = nc.dram_tensor("cst", [128, CW], _f16, kind="ExternalInput")
    y = nc.dram_tensor("y", [H2, N_TILES * BT], _f16, kind="ExternalOutput")

    with SplitDrainTileContext(nc) as tc:
        with (
            tc.tile_pool(name="consts", bufs=1) as cpool,
            tc.tile_pool(name="hp", bufs=4) as hpool,
            tc.tile_pool(name="psh", bufs=4, space="PSUM") as psh,
            tc.tile_pool(name="pso", bufs=3, space="PSUM") as pso,
            tc.tile_pool(name="wps", bufs=1, space="PSUM") as wpool,
        ):
            cst_sb = cpool.tile([128, CW], _f16, tag="cst")
            w1m_sb = cst_sb[:, : FC * H1]
            # two masked variants, each replicated at both 32-row bands so
            # weights and fmap share a start partition (walrus requires it):
            # lo cols: [w1rem; 0] pattern, hi cols: [0; w1rem]
            w1r_sb = cst_sb[:, FC * H1 : FC * H1 + 2 * H1]
            w2_sb = cst_sb[:H1, FC * H1 + 2 * H1 : FC * H1 + 2 * H1 + H2]
            # biases ride the f16 pack; DVE upcasts them to f32 once
            # (tensor_scalar requires f32 scalar operands)
            bias_sb = cpool.tile([H1, 2], _f32, tag="bias")
            b1_sb = bias_sb[:H1, 0:1]
            b2_sb = bias_sb[:H2, 1:2]
            xr_sb = cpool.tile([64, 4 * BT], _f16, tag="xr")
            # outputs accumulate here; tapered writeback
            o_sb = cpool.tile([H2, N_TILES * BT], _f16, tag="o")
            # the whole x shard lives in SBUF: no buffer reuse, no WAR
            # stalls; per-slice RAW deps gate each tile's matmuls.
            xm_sb = cpool.tile([128, N_TILES * TW], _f16, tag="xm")

            # DMA schedule: whole-tile transfers (786 KB; smaller DMAs
            # measurably drop aggregate ring bandwidth ~25%), alternating
            # rings so tiles land in processing order at ~1.9us cadence:
            #   A (sync):   cst, t1, xr_b0, t3, xr_b1, t5, t7, ..., t15
            #   B (scalar): t0, t2, t4, ..., t14
            # xr bands are slotted just-in-time: band 0 (tiles 0-7) right
            # after t1, band 1 (tiles 8-15) after t3.
            nc.scalar.dma_start(out=xm_sb[:, :TW], in_=xm[:, :TW])
            nc.sync.dma_start(out=cst_sb[:], in_=cst[:])
            for t in range(1, N_TILES):
                c0 = t * TW
                eng = nc.sync if t % 2 == 1 else nc.scalar
                eng.dma_start(out=xm_sb[:, c0 : c0 + TW], in_=xm[:, c0 : c0 + TW])
                if t == 1:
                    nc.sync.dma_start(
                        out=xr_sb[:32, :], in_=xr[:32, :]
                    )
                elif t == 3:
                    nc.sync.dma_start(
                        out=xr_sb[32:, :], in_=xr[32:, :]
                    )
            nc.vector.tensor_copy(bias_sb[:], cst_sb[:H1, CW - 2 : CW])

            # PE pre-warm while the first tile streams in (~2us): keeps the
            # clock/HAM state hot so the real stream runs at full rate.
            warm_sb = cpool.tile([128, 64], _f16, tag="warm")
            nc.vector.memset(warm_sb[:], 0)
            warm_ps = wpool.tile([64, 64], _f32, tag="wps")
            for _ in range(warm_iters):
                nc.tensor.matmul(
                    warm_ps[:], warm_sb[:, :64], warm_sb[:, :64], start=True, stop=True
                )

            # tile-serial stream, software-pipelined one deep: fc2 of tile
            # t-1 is emitted AFTER tile t's layer-1 matmuls so the PE never
            # waits on DVE's relu latency (~0.9us) — relu(t-1) completes
            # while the PE streams tile t. LDWEIGHTS is emitted per-matmul
            # anyway (no stationary reuse in the ISA), so tile pairing buys
            # nothing over single-tile processing.
            # tapered writeback: big chunks leave mid-kernel, the final
            # write after the last tile is two tiles (20 KB f16)
            flush = {7: [(0, 8)], 11: [(8, 12)], 13: [(12, 14)],
                     15: [(14, 16)]}

            def fc2_and_out(tt, h_prev):
                po = pso.tile([H2, BT], _f32, tag="po", name="po")
                nc.tensor.matmul(po[:], w2_sb[:], h_prev[:], start=True, stop=True)
                nc.vector.tensor_scalar_add(
                    o_sb[:, tt * BT : (tt + 1) * BT], po[:], b2_sb[:, 0:1]
                )
                for i, (a, b) in enumerate(flush.get(tt, [])):
                    eng = nc.sync if (tt + i) % 2 == 1 else nc.scalar
                    eng.dma_start(
                        out=y[:, a * BT : b * BT], in_=o_sb[:, a * BT : b * BT]
                    )

            prev = None
            for t in range(N_TILES):
                xs_ = xm_sb[:, t * TW : (t + 1) * TW]
                g, q = t // 4, t % 4
                band, variant = g // 2, g % 2
                ph = psh.tile([H1, BT], _f32, tag="ph", name="ph")
                for c in range(FC):
                    nc.tensor.matmul(
                        ph[:],
                        w1m_sb[:, c * H1 : (c + 1) * H1],
                        xs_[:, c * BT : (c + 1) * BT],
                        start=(c == 0),
                        stop=False,
                    )
                # tight-packed remainder: the 32-row band holds two groups;
                # the w1r variant zero-masks the half that isn't group g.
                nc.tensor.matmul(
                    ph[:],
                    w1r_sb[32 * band : 32 * band + 32, variant * H1 : variant * H1 + H1],
                    xr_sb[32 * band : 32 * band + 32, q * BT : (q + 1) * BT],
                    start=False,
                    stop=True,
                    tile_position=(32 * band, 0),
                )

                # relu(ph + b1) on DVE — ACT stays a pure DMA-issue engine so
                # its HWDGE ring never stalls behind compute
                h = hpool.tile([H1, BT], _f16, tag="h", name="h")
                nc.vector.tensor_scalar(
                    h[:],
                    ph[:],
                    b1_sb[:, 0:1],
                    0.0,
                    mybir.AluOpType.add,
                    mybir.AluOpType.max,
                )
                if prev is not None:
                    fc2_and_out(*prev)
                prev = (t, h)
            fc2_and_out(*prev)

    _split_sync_waits(nc)
    return nc


def _fold_conv_fc1(conv_w: np.ndarray, fc1_w: np.ndarray) -> np.ndarray:
    """Fold the 3x3 valid conv into fc1: W1eff[784, 100] such that
    h = x @ W1eff  ==  fc1( flatten(conv(x)) ).  Accumulated in float64."""
    F = fc1_w.astype(np.float64).T.reshape(26, 26, H1)
    W = np.zeros((28, 28, H1), np.float64)
    cw = conv_w.astype(np.float64)
    for di in range(3):
        for dj in range(3):
            W[di : di + 26, dj : dj + 26, :] += cw[di, dj] * F
    return W.reshape(784, H1).astype(np.float32)


def _make_in_maps(x, conv_w, fc1_w, fc1_b, fc2_w, fc2_b):
    w1eff = _fold_conv_fc1(conv_w, fc1_w)
    w1m = np.ascontiguousarray(
        w1eff[: FC * 128]
        .astype(np.float16)
        .reshape(FC, 128, H1)
        .transpose(1, 0, 2)
        .reshape(128, FC * H1)
    )
    # lo cols: w1rem at band rows 0-15 (zeros at 16-31); hi cols: w1rem at
    # band rows 16-31; both replicated at bands 0-31 and 32-63
    w1rem = w1eff[FC * 128 :].astype(np.float16)
    w1r = np.zeros((128, 2 * H1), np.float16)
    for band in range(2):
        w1r[32 * band : 32 * band + F_REM, :H1] = w1rem
        w1r[32 * band + F_REM : 32 * band + 32, H1:] = w1rem
    cst = np.zeros((128, CW), np.float16)
    cst[:, : FC * H1] = w1m
    cst[:, FC * H1 : FC * H1 + 2 * H1] = w1r
    cst[:H1, FC * H1 + 2 * H1 : FC * H1 + 2 * H1 + H2] = fc2_w.T.astype(np.float16)
    cst[:H1, CW - 2] = fc1_b.astype(np.float16)
    cst[:H2, CW - 1] = fc2_b.astype(np.float16)

    in_maps = []
    for s in range(N_CORES):
        xs = x[s * B_SHARD : (s + 1) * B_SHARD].reshape(N_TILES, BT, 784)
        # xm[p, t*TW + c*BT + b] = xs[t, b, c*128 + p]
        xm = np.ascontiguousarray(
            xs[:, :, : FC * 128]
            .astype(np.float16)
            .reshape(N_TILES, BT, FC, 128)
            .transpose(3, 0, 2, 1)
            .reshape(128, N_TILES * TW)
        )
        xr_flat = xs.reshape(B_SHARD, 784)[:, FC * 128 :].astype(np.float16)
        # tight pack: band b holds groups 2b (rows 0-15 of the band) and
        # 2b+1 (rows 16-31); columns index the group's 4 tiles
        xr = np.zeros((64, 4 * BT), np.float16)
        for t in range(N_TILES):
            g, q = t // 4, t % 4
            r0 = 32 * (g // 2) + F_REM * (g % 2)
            xr[r0 : r0 + F_REM, q * BT : (q + 1) * BT] = xr_flat[
                t * BT : (t + 1) * BT
            ].T
        in_maps.append({"xm": xm, "xr": xr, "cst": cst})
    return in_maps


def _gather(results) -> np.ndarray:
    out = np.empty((B_TOTAL, H2), np.float32)
    for s in range(N_CORES):
        ys = results[s]["y"]  # [H2, B_SHARD] f16
        out[s * B_SHARD : (s + 1) * B_SHARD] = ys.T.astype(np.float32)
    return out


def kernel_run(inputs: dict, trace: bool = False):
    """Run the kernel; returns (full output (65536,10) f32, BassKernelResults)."""
    x = np.ascontiguousarray(np.asarray(inputs["x"], dtype=np.float32))
    assert x.shape == (B_TOTAL, 784), x.shape
    in_maps = _make_in_maps(
        x,
        np.asarray(inputs["conv_w"], np.float32),
        np.asarray(inputs["fc1_w"], np.float32),
        np.asarray(inputs["fc1_b"], np.float32),
        np.asarray(inputs["fc2_w"], np.float32),
        np.asarray(inputs["fc2_b"], np.float32),
    )
    nc = _build_nc()
    res = run_bass_kernel_spmd(nc, in_maps, core_ids=list(range(N_CORES)), trace=trace)
    return _gather(res.results), res


def kernel(**inputs) -> np.ndarray:
    out, _ = kernel_run(inputs)
    return out


# revision 22
# speedup vs baseline: 1.0721x; 1.0098x over previous
"""Trainium2 Bass kernel for DigitConvolutionalModel (8-core data parallel).

Computation: x(B,784) -> 3x3 valid conv on 28x28 -> flatten(676)
             -> FC(100)+ReLU -> FC(10), B = 65536.

Algebraic restructure (host side, exact): the conv is linear, so conv and
fc1 fold into one 784->100 matrix W1eff (accumulated in float64). The
device kernel is then just two matmul layers per 512-sample tile:
  h = relu(x @ W1eff + b1);  y = h @ fc2_w.T + b2.

Numerics: the matmul datapath runs in fp16 (inputs rounded once on the
host); the output writeback is fp16 too (upcast on host). Measured
end-to-end scale-relative absmax error vs the fp32 reference ~9e-4.

Per-core layout (B_shard=8192 = 16 tiles x 512): x is pre-transposed on
the host to feature-major so the contraction lands on SBUF partitions.
The WHOLE shard (12.6 MB) is SBUF-resident: per tile one [128, 6*512]
fp16 slab (features 0..767), remainders packed in xr as in the baseline.
The x stream is 16 whole-tile DMAs balanced across the two HWDGE rings
(sync carries w1m+xr+consts then odd tiles; scalar carries even tiles,
tile 0 split in half so the PE can start early). With no buffer reuse
there are no WAR hazards: the SDMA engines stream HBM at line rate
start to finish, and the PE (2.06us/tile pair cadence) tracks ~1 tile
behind the DMA stream (2.2us/tile) instead of queueing behind an
oversized warmup.
"""

import numpy as np

import concourse.bass as bass
import concourse.mybir as mybir
import concourse.tile as tile
from concourse.bass_utils import run_bass_kernel_spmd
from concourse.vector_clock import ScopedClock

N_CORES = 8
B_TOTAL = 65536
B_SHARD = B_TOTAL // N_CORES  # 8192
BT = 512  # batch tile (one PSUM bank of fp32)
N_TILES = B_SHARD // BT  # 16
FC = 6  # full 128-partition feature chunks (6*128 = 768)
F_REM = 784 - FC * 128  # 16 remainder features
H1 = 100
H2 = 10
TW = FC * BT  # columns of one tile in the xm slab (3072)

_f32 = mybir.dt.float32
_f16 = mybir.dt.float16


class SplitDrainTileContext(tile.TileContext):
    """TileContext whose tail drain carries at most one sync wait.

    The pinned walrus rejects instructions with >2 sync waits
    ("Too many sync wait commands" in setupSyncWait); the stock tail
    drain accumulates one wait per active proc. Emit one drain per
    wait instead — consecutive drains on the sync engine are
    semantically equivalent to one drain carrying all the waits.
    """

    def _drain_and_barrier(self, tick_clock, wait_clock):
        nc = self.nc
        # Cheap tail: the stock version runs two full EVSEM butterflies
        # (~13us measured). Instead: gpsimd waits on the whole vector
        # clock (all tracked incs have landed), every engine drains its
        # own DGE queues, gpsimd clears the sem ranges, and one
        # sequencer-level sem-only barrier closes the kernel.
        drain_inst = nc.gpsimd.drain()
        wait_clock.add_sem_waits(
            drain_inst.ins, ScopedClock({None: tick_clock.global_clock})
        )
        raw = drain_inst.ins
        si = raw.sync_info
        if si is not None and si.on_wait and len(si.on_wait) > 1:
            waits = list(si.on_wait)
            si.on_wait = waits[:1]
            raw.sync_info = si
            for w in waits[1:]:
                extra = nc.gpsimd.drain()
                extra.ins.sync_info = mybir.SyncInfo(on_wait=[w], on_update=[])
        for eng in (nc.sync, nc.scalar, nc.vector, nc.tensor):
            eng.drain()

        # No tail barrier: gpsimd's global-clock waits above guarantee all
        # tracked sem incs (incl. DMA completions) have landed before the
        # clears, and NRT serializes re-executions on all-engine completion.
        assert self.sems is not None
        popped = nc._tile_sem_poison_stack.pop()
        assert popped is self._sem_poison
        nc.clear_and_free_semaphores(list(self.sems.allocated().values()))


def _split_sync_waits(nc: bass.Bass, limit: int = 1) -> None:
    """Walrus-compat post-pass: the pinned walrus rejects instructions
    carrying more than ~2 sync waits. Hoist excess waits onto NoOp
    instructions inserted just before the offending instruction on the
    same engine — semantically identical (waits run in stream order)."""
    n = 0
    for fn in nc.m.functions:
        for bb in fn.blocks:
            out = []
            changed = False
            for inst in bb.instructions:
                si = inst.sync_info
                if si is not None and si.on_wait and len(si.on_wait) > limit:
                    waits = list(si.on_wait)
                    for i in range(0, len(waits) - limit, limit):
                        nop = mybir.InstNoOp(
                            name=f"swsplit-{n}",
                            ins=[],
                            outs=[],
                            sync_info=mybir.SyncInfo(
                                on_wait=waits[i : i + limit], on_update=[]
                            ),
                        )
                        nop.engine = inst.engine
                        out.append(nop)
                        n += 1
                    si.on_wait = waits[len(waits) - limit :]
                    inst.sync_info = si
                    changed = True
                out.append(inst)
            if changed:
                bb.instructions = out
    return


# packed consts cols: w1m 600 | w1r lo 100 | w1r hi 100 | w2 10 | b1 1 | b2 1
CW = FC * H1 + 2 * H1 + H2 + 2


def _build_nc(warm_iters: int = 92) -> bass.Bass:
    nc = bass.Bass(monotonic_sem_count=0)
    # whole shard, feature-major: column index = t*TW + c*BT + b
    xm = nc.dram_tensor("xm", [128, N_TILES * TW], _f16, kind="ExternalInput")
    # remainder features, tight-packed: band 0 (rows 0-31) holds groups
    # 0 (rows 0-15) and 1 (rows 16-31); band 1 (rows 32-63) holds groups
    # 2 and 3. The unwanted half of each band is masked by zeros in the
    # stationary (w1r lo/hi variants), so no padding bytes ride the DMA.
    xr = nc.dram_tensor("xr", [64, 4 * BT], _f16, kind="ExternalInput")
    # all weights/biases packed in ONE tensor so the preload is cheap to
    # issue (each HWDGE issue costs ~0.6us of ring-issue time)
    cst = nc.dram_tensor("cst", [128, CW], _f16, kind="ExternalInput")
    y = nc.dram_tensor("y", [H2, N_TILES * BT], _f16, kind="ExternalOutput")

    with SplitDrainTileContext(nc) as tc:
        with (
            tc.tile_pool(name="consts", bufs=1) as cpool,
            tc.tile_pool(name="hp", bufs=4) as hpool,
            tc.tile_pool(name="psh", bufs=4, space="PSUM") as psh,
            tc.tile_pool(name="pso", bufs=3, space="PSUM") as pso,
            tc.tile_pool(name="wps", bufs=1, space="PSUM") as wpool,
        ):
            cst_sb = cpool.tile([128, CW], _f16, tag="cst")
            w1m_sb = cst_sb[:, : FC * H1]
            # two masked variants, each replicated at both 32-row bands so
            # weights and fmap share a start partition (walrus requires it):
            # lo cols: [w1rem; 0] pattern, hi cols: [0; w1rem]
            w1r_sb = cst_sb[:, FC * H1 : FC * H1 + 2 * H1]
            w2_sb = cst_sb[:H1, FC * H1 + 2 * H1 : FC * H1 + 2 * H1 + H2]
            # biases ride the f16 pack; DVE upcasts them to f32 once
            # (tensor_scalar requires f32 scalar operands)
            bias_sb = cpool.tile([H1, 2], _f32, tag="bias")
            b1_sb = bias_sb[:H1, 0:1]
            b2_sb = bias_sb[:H2, 1:2]
            xr_sb = cpool.tile([64, 4 * BT], _f16, tag="xr")
            # outputs accumulate here; tapered writeback
            o_sb = cpool.tile([H2, N_TILES * BT], _f16, tag="o")
            # the whole x shard lives in SBUF: no buffer reuse, no WAR
            # stalls; per-slice RAW deps gate each tile's matmuls.
            xm_sb = cpool.tile([128, N_TILES * TW], _f16, tag="xm")

            # DMA schedule: whole-tile transfers (786 KB; smaller DMAs
            # measurably drop aggregate ring bandwidth ~25%), alternating
            # rings so tiles land in processing order at ~1.9us cadence:
            #   A (sync):   cst, t1, xr_b0, t3, xr_b1, t5, t7, ..., t15
            #   B (scalar): t0, t2, t4, ..., t14
            # xr bands are slotted just-in-time: band 0 (tiles 0-7) right
            # after t1, band 1 (tiles 8-15) after t3.
            nc.scalar.dma_start(out=xm_sb[:, :TW], in_=xm[:, :TW])
            nc.sync.dma_start(out=cst_sb[:], in_=cst[:])
            for t in range(1, N_TILES):
                c0 = t * TW
                eng = nc.sync if t % 2 == 1 else nc.scalar
                eng.dma_start(out=xm_sb[:, c0 : c0 + TW], in_=xm[:, c0 : c0 + TW])
                if t == 1:
                    nc.sync.dma_start(
                        out=xr_sb[:32, :], in_=xr[:32, :]
                    )
                elif t == 3:
                    nc.sync.dma_start(
                        out=xr_sb[32:, :], in_=xr[32:, :]
                    )
            nc.vector.tensor_copy(bias_sb[:], cst_sb[:H1, CW - 2 : CW])

            # PE pre-warm while the first tile streams in: HAM grants the
            # full-clock duty cycle after ~3.5-4.5us of sustained LIGHT
            # activity and hysteresis then holds the grant through the
            # heavy stream; starting the heavy stream cold leaves the PE at
            # half clock for >12us. Sized to bridge, gap-free, from the
            # preamble end (~7.4us) to tile 0 ready (~12.3us): 92 x 53ns.
            warm_sb = cpool.tile([128, 64], _f16, tag="warm")
            nc.vector.memset(warm_sb[:], 0)
            warm_ps = wpool.tile([64, 64], _f32, tag="wps")
            for _ in range(warm_iters):
                nc.tensor.matmul(
                    warm_ps[:], warm_sb[:, :64], warm_sb[:, :64], start=True, stop=True
                )

            # tile-serial stream, software-pipelined one deep: fc2 of tile
            # t-1 is emitted AFTER tile t's layer-1 matmuls so the PE never
            # waits on DVE's relu latency (~0.9us) — relu(t-1) completes
            # while the PE streams tile t. LDWEIGHTS is emitted per-matmul
            # anyway (no stationary reuse in the ISA), so tile pairing buys
            # nothing over single-tile processing.
            # tapered writeback: big chunks leave mid-kernel, the final
            # write after the last tile is two tiles (20 KB f16)
            flush = {7: [(0, 8)], 11: [(8, 12)], 13: [(12, 14)],
                     15: [(14, 16)]}

            def fc2_and_out(tt, h_prev):
                po = pso.tile([H2, BT], _f32, tag="po", name="po")
                nc.tensor.matmul(po[:], w2_sb[:], h_prev[:], start=True, stop=True)
                nc.vector.tensor_scalar_add(
                    o_sb[:, tt * BT : (tt + 1) * BT], po[:], b2_sb[:, 0:1]
                )
                for i, (a, b) in enumerate(flush.get(tt, [])):
                    eng = nc.sync if (tt + i) % 2 == 1 else nc.scalar
                    eng.dma_start(
                        out=y[:, a * BT : b * BT], in_=o_sb[:, a * BT : b * BT]
                    )

            prev = None
            for t in range(N_TILES):
                xs_ = xm_sb[:, t * TW : (t + 1) * TW]
                g, q = t // 4, t % 4
                band, variant = g // 2, g % 2
                ph = psh.tile([H1, BT], _f32, tag="ph", name="ph")
                for c in range(FC):
                    nc.tensor.matmul(
                        ph[:],
                        w1m_sb[:, c * H1 : (c + 1) * H1],
                        xs_[:, c * BT : (c + 1) * BT],
                        start=(c == 0),
                        stop=False,
                    )
                # tight-packed remainder: the 32-row band holds two groups;
                # the w1r variant zero-masks the half that isn't group g.
                nc.tensor.matmul(
                    ph[:],
                    w1r_sb[32 * band : 32 * band + 32, variant * H1 : variant * H1 + H1],
                    xr_sb[32 * band : 32 * band + 32, q * BT : (q + 1) * BT],
                    start=False,
                    stop=True,
                    tile_position=(32 * band, 0),
                )

                # relu(ph + b1) on DVE — ACT stays a pure DMA-issue engine so
                # its HWDGE ring never stalls behind compute
                h = hpool.tile([H1, BT], _f16, tag="h", name="h")
                nc.vector.tensor_scalar(
                    h[:],
                    ph[:],
                    b1_sb[:, 0:1],
                    0.0,
                    mybir.AluOpType.add,
                    mybir.AluOpType.max,
                )
                if prev is not None:
                    fc2_and_out(*prev)
                prev = (t, h)
            fc2_and_out(*prev)

    _split_sync_waits(nc)
    return nc


def _fold_conv_fc1(conv_w: np.ndarray, fc1_w: np.ndarray) -> np.ndarray:
    """Fold the 3x3 valid conv into fc1: W1eff[784, 100] such that
    h = x @ W1eff  ==  fc1( flatten(conv(x)) ).  Accumulated in float64."""
    F = fc1_w.astype(np.float64).T.reshape(26, 26, H1)
    W = np.zeros((28, 28, H1), np.float64)
    cw = conv_w.astype(np.float64)
    for di in range(3):
        for dj in range(3):
            W[di : di + 26, dj : dj + 26, :] += cw[di, dj] * F
    return W.reshape(784, H1).astype(np.float32)


def _make_in_maps(x, conv_w, fc1_w, fc1_b, fc2_w, fc2_b):
    w1eff = _fold_conv_fc1(conv_w, fc1_w)
    w1m = np.ascontiguousarray(
        w1eff[: FC * 128]
        .astype(np.float16)
        .reshape(FC, 128, H1)
        .transpose(1, 0, 2)
        .reshape(128, FC * H1)
    )
    # lo cols: w1rem at band rows 0-15 (zeros at 16-31); hi cols: w1rem at
    # band rows 16-31; both replicated at bands 0-31 and 32-63
    w1rem = w1eff[FC * 128 :].astype(np.float16)
    w1r = np.zeros((128, 2 * H1), np.float16)
    for band in range(2):
        w1r[32 * band : 32 * band + F_REM, :H1] = w1rem
        w1r[32 * band + F_REM : 32 * band + 32, H1:] = w1rem
    cst = np.zeros((128, CW), np.float16)
    cst[:, : FC * H1] = w1m
    cst[:, FC * H1 : FC * H1 + 2 * H1] = w1r
    cst[:H1, FC * H1 + 2 * H1 : FC * H1 + 2 * H1 + H2] = fc2_w.T.astype(np.float16)
    cst[:H1, CW - 2] = fc1_b.astype(np.float16)
    cst[:H2, CW - 1] = fc2_b.astype(np.float16)

    in_maps = []
    for s in range(N_CORES):
        xs = x[s * B_SHARD : (s + 1) * B_SHARD].reshape(N_TILES, BT, 784)
        # xm[p, t*TW + c*BT + b] = xs[t, b, c*128 + p]
        xm = np.ascontiguousarray(
            xs[:, :, : FC * 128]
            .astype(np.float16)
            .reshape(N_TILES, BT, FC, 128)
            .transpose(3, 0, 2, 1)
            .reshape(128, N_TILES * TW)
        )
        xr_flat = xs.reshape(B_SHARD, 784)[:, FC * 128 :].astype(np.float16)
        # tight pack: band b holds groups 2b (rows 0-15 of the band) and
        # 2b+1 (rows 16-31); columns index the group's 4 tiles
        xr = np.zeros((64, 4 * BT), np.float16)
        for t in range(N_TILES):
            g, q = t // 4, t % 4
            r0 = 32 * (g // 2) + F_REM * (g % 2)
            xr[r0 : r0 + F_REM, q * BT : (q + 1) * BT] = xr_flat[
                t * BT : (t + 1) * BT
            ].T
        in_maps.append({"xm": xm, "xr": xr, "cst": cst})
    return in_maps


def _gather(results) -> np.ndarray:
    out = np.empty((B_TOTAL, H2), np.float32)
    for s in range(N_CORES):
        ys = results[s]["y"]  # [H2, B_SHARD] f16
        out[s * B_SHARD : (s + 1) * B_SHARD] = ys.T.astype(np.float32)
    return out


def kernel_run(inputs: dict, trace: bool = False):
    """Run the kernel; returns (full output (65536,10) f32, BassKernelResults)."""
    x = np.ascontiguousarray(np.asarray(inputs["x"], dtype=np.float32))
    assert x.shape == (B_TOTAL, 784), x.shape
    in_maps = _make_in_maps(
        x,
        np.asarray(inputs["conv_w"], np.float32),
        np.asarray(inputs["fc1_w"], np.float32),
        np.asarray(inputs["fc1_b"], np.float32),
        np.asarray(inputs["fc2_w"], np.float32),
        np.asarray(inputs["fc2_b"], np.float32),
    )
    nc = _build_nc()
    res = run_bass_kernel_spmd(nc, in_maps, core_ids=list(range(N_CORES)), trace=trace)
    return _gather(res.results), res


def kernel(**inputs) -> np.ndarray:
    out, _ = kernel_run(inputs)
    return out
